# revision 28
# baseline (speedup 1.0000x reference)
"""GATv2 actor layer (nn_GATv2Actor) on 8 TRN2 NeuronCores via Bass/Tile. v2.

Self-contained: kernel(**inputs) takes the full (unsharded) inputs of
reference.setup_inputs() and returns the full [50000, 4] float32 output.

Distribution strategy (edge-parallel by destination-node range):
  - node n is owned by core n // 6250; each core handles all edges whose
    destination lies in its range (plus its self-loops), so the segment
    softmax and the scatter-add are fully core-local and the final
    output rows are disjoint (host just concatenates - no collective).

v2 changes vs the 897us baseline (CoreSim-derived):
  - attention weight vector attn_w is folded into the pair tables host-side
    (columns scaled by attn and sign-permuted), so per-edge logits become
    lg = sum_d leakyish(t_d) with t = a_src[src] + a_dst[dst]; leaky
    handled per sign group: pos cols max(0.2t, t), neg cols min(0.2t, t).
  - the dst-node one-hot (for the scatter matmul) is *embedded in the
    adst table* and fetched by the same gather - no is_equal build.
  - gathers fetch f32-typed rows (2x fewer billed elements in the
    Pool-engine cost model), bitcast back to fp16 in SBUF; the dst
    one-hot is a host-precomputed array streamed on the SP DMA queue.
  - logit reduce = 4 binary folds (fp16, 2x DVE) + one small reduce.
  - exp is written by the Activation engine directly into cols 128:130 of
    the matmul moving operand; a width-4 exp replica broadcast on a
    middle axis feeds the value-scale multiply (keeps DVE 2x mode).
  - work is spread across DVE / Pool / Act / SP / PE so no engine is
    >~55% of the total (baseline: DVE at 96.9%).

SPMD uniformity: one program runs on all 8 cores; per-(block,stream)
tile counts are padded to the max over cores. int16 gather indices
limit tables to 32767 rows, so edges are split into two streams by
src < 32768 gathering from two base offsets of the asv table.
"""
import sys

import numpy as np

sys.path.insert(0, "/opt/trn_rl_repo")

import concourse.bass as bass  # noqa: E402
import concourse.tile as tile  # noqa: E402
from concourse import bacc, mybir  # noqa: E402
from concourse.bass_utils import run_bass_kernel_spmd  # noqa: E402

FP16 = mybir.dt.float16
F32 = mybir.dt.float32
I16 = mybir.dt.int16
U32 = mybir.dt.float32  # gather packing dtype (u64 broken on backend)
AT = mybir.AluOpType
ACTF = mybir.ActivationFunctionType

F = 128      # feature dim
H = 2        # heads
D = 64       # head dim
P_OUT = 4    # phases
N_CORES = 8


def prep(h_int, edge_index, pair_W, pair_b, attn_w, value_W, out_W, out_b,
         phase_W, phase_b, n_cores=N_CORES, G=24, split=32768, A_CH=16):
    """Host-side index preprocessing + input packing. Returns (meta, in_maps)."""
    h = np.asarray(h_int, np.float32)
    ei = np.asarray(edge_index)
    pair_W = np.asarray(pair_W, np.float32)
    pair_b = np.asarray(pair_b, np.float32)
    attn_w = np.asarray(attn_w, np.float32)
    value_W = np.asarray(value_W, np.float32)
    out_W = np.asarray(out_W, np.float32)
    out_b = np.asarray(out_b, np.float32)
    phase_W = np.asarray(phase_W, np.float32)
    phase_b = np.asarray(phase_b, np.float32)
    N = h.shape[0]
    assert N % n_cores == 0
    NPC = N // n_cores
    NBLK = (NPC + 127) // 128
    NPAD = ((N + 127) // 128) * 128
    ZROW = NBLK * 128          # adst pad row (zero one-hot)
    assert NPAD - split < 32768 and split < 32768 + 1

    src = np.concatenate([ei[0], np.arange(N)]).astype(np.int64)
    dst = np.concatenate([ei[1], np.arange(N)]).astype(np.int64)
    core = dst // NPC

    percore = []
    counts = np.zeros((n_cores, 2, NBLK), np.int64)
    for c in range(n_cores):
        m = core == c
        es = src[m]
        ed = dst[m] - c * NPC
        o = np.lexsort((es, ed))
        es, ed = es[o], ed[o]
        lo = es < split
        percore.append((es, ed, lo))
        for si in range(2):
            msk = lo if si == 0 else ~lo
            counts[c, si] = np.bincount(ed[msk] // 128, minlength=NBLK)
    T = np.ceil(counts.max(axis=0) / 128.0).astype(np.int64)  # [2, NBLK]
    tiles = T.sum(axis=1)
    L = tiles * 128
    base_tile = np.zeros((2, NBLK + 1), np.int64)
    base_tile[:, 1:] = np.cumsum(T, axis=1)

    f16 = np.float16
    # --- attn folding + per-head sign permutation ---------------------
    # col order per head: attn>=0 columns first. Tables' "a" halves are
    # pre-scaled by attn so lg = sum_d leaky-in-t-space(t_d).
    perm = []
    cpos = []
    for hh in range(H):
        a = attn_w[hh]
        p = np.argsort(a < 0, kind="stable")  # pos cols first
        perm.append(p)
        cpos.append(int((a >= 0).sum()))
    W_src_s = np.concatenate(
        [pair_W[hh, :F][:, perm[hh]] * attn_w[hh, perm[hh]] for hh in range(H)],
        axis=1)                                            # [F, 128]
    W_dst_s = np.concatenate(
        [pair_W[hh, F:][:, perm[hh]] * attn_w[hh, perm[hh]] for hh in range(H)],
        axis=1)                                            # [F, 128]
    b_s = np.concatenate(
        [pair_b[hh, perm[hh]] * attn_w[hh, perm[hh]] for hh in range(H)])

    W_asv2 = np.concatenate(
        [W_src_s, value_W[0], value_W[1]], axis=1).astype(f16)   # [128, 256]
    W_dst2 = W_dst_s.astype(f16)                                 # [128, 128]
    bias_bc = np.broadcast_to(b_s.astype(f16), (128, F)).copy()
    ident16 = np.eye(128, dtype=f16)
    ident = np.eye(128, dtype=np.float32)
    out_Wt = np.asarray(out_W, f16)
    out_b_c = np.asarray(out_b, np.float32).reshape(128, 1).copy()
    phase_Wt = np.asarray(phase_W, f16)
    phase_b_bc = np.broadcast_to(np.asarray(phase_b, f16),
                                 (128, P_OUT)).copy()

    hp = np.zeros((NPAD, F), np.float32)
    hp[:N] = h
    hT16 = np.ascontiguousarray(hp.T.astype(f16))

    shared = dict(hT16=hT16, W_asv2=W_asv2, W_dst2=W_dst2, bias_bc=bias_bc,
                  ident16=ident16, ident=ident, out_Wt=out_Wt,
                  out_b=out_b_c, phase_Wt=phase_Wt, phase_b_bc=phase_b_bc)

    in_maps = []
    for c in range(n_cores):
        es, ed, lo = percore[c]
        m = {}
        for si in range(2):
            msk = lo if si == 0 else ~lo
            es_s, ed_s = es[msk], ed[msk]
            gidx = np.zeros(L[si], np.int16)
            dloc = np.full(L[si], ZROW, np.int16)
            starts = np.searchsorted(ed_s, np.arange(NBLK + 1) * 128)
            for j in range(NBLK):
                seg = slice(starts[j], starts[j + 1])
                n = starts[j + 1] - starts[j]
                b0 = base_tile[si, j] * 128
                gidx[b0:b0 + n] = (es_s[seg] - (split if si else 0)).astype(np.int16)
                dloc[b0:b0 + n] = ed_s[seg].astype(np.int16)
            m[f"gw{si}"] = np.tile(gidx.reshape(-1, 16).T, (8, 1)).copy()
            m[f"dw{si}"] = np.tile(dloc.reshape(-1, 16).T, (8, 1)).copy()
            oh_arr = np.zeros((128, L[si]), f16)
            idx = np.arange(L[si])
            real = dloc != ZROW
            oh_arr[idx[real] % 128,
                   (idx[real] // 128) * 128 + (dloc[real].astype(np.int64) % 128)] = 1.0
            m[f"oh{si}"] = oh_arr
        hl = np.zeros((NBLK * 128, F), np.float32)
        hl[:NPC] = h[c * NPC:(c + 1) * NPC]
        m["hlocT16"] = np.ascontiguousarray(hl.T.astype(f16))
        m.update(shared)
        in_maps.append(m)

    meta = dict(N=N, NPC=NPC, NBLK=NBLK, NPAD=NPAD, split=split, G=G,
                A_CH=A_CH, T=T, tiles=tiles, L=L, base_tile=base_tile,
                cpos=cpos, n_cores=n_cores, ZROW=ZROW)
    return meta, in_maps


def build(meta):
    NPC, NBLK, NPAD = meta["NPC"], meta["NBLK"], meta["NPAD"]
    split, G, A_CH = meta["split"], meta["G"], meta["A_CH"]
    T, tiles, L = meta["T"], meta["tiles"], meta["L"]
    base_tile = meta["base_tile"]
    cpos = meta["cpos"]
    ADST_ROWS = NBLK * 128 + 128   # + zero pad rows
    last_rows = NPC - (NBLK - 1) * 128

    nc = bacc.Bacc(None, target_bir_lowering=False, debug=False)

    hT_d = nc.dram_tensor("hT16", [128, NPAD], FP16, kind="ExternalInput")
    hloc_d = nc.dram_tensor("hlocT16", [128, NBLK * 128], FP16,
                            kind="ExternalInput")
    gw_d = [nc.dram_tensor(f"gw{s}", [128, int(L[s]) // 16], I16,
                           kind="ExternalInput") for s in range(2)]
    oh_d = [nc.dram_tensor(f"oh{s}", [128, int(L[s])], FP16,
                           kind="ExternalInput") for s in range(2)]
    dw_d = [nc.dram_tensor(f"dw{s}", [128, int(L[s]) // 16], I16,
                           kind="ExternalInput") for s in range(2)]
    Wasv_d = nc.dram_tensor("W_asv2", [128, 256], FP16, kind="ExternalInput")
    Wdst_d = nc.dram_tensor("W_dst2", [128, 128], FP16, kind="ExternalInput")
    bias_d = nc.dram_tensor("bias_bc", [128, 128], FP16, kind="ExternalInput")
    id16_d = nc.dram_tensor("ident16", [128, 128], FP16, kind="ExternalInput")
    ident_d = nc.dram_tensor("ident", [128, 128], F32, kind="ExternalInput")
    outW_d = nc.dram_tensor("out_Wt", [128, 128], FP16, kind="ExternalInput")
    outb_d = nc.dram_tensor("out_b", [128, 1], F32, kind="ExternalInput")
    phW_d = nc.dram_tensor("phase_Wt", [128, P_OUT], FP16, kind="ExternalInput")
    phb_d = nc.dram_tensor("phase_b_bc", [128, P_OUT], FP16, kind="ExternalInput")

    asv_d = nc.dram_tensor("asv_tab", [NPAD, 256], FP16)
    adst_d = nc.dram_tensor("adst_tab", [ADST_ROWS, 128], FP16)
    out_d = nc.dram_tensor("out", [NPC, P_OUT], F32, kind="ExternalOutput")

    with tile.TileContext(nc) as tc:
        with tc.tile_pool(name="consts", bufs=1) as pc:
            def cload(name, dram, shape, dtype):
                t = pc.tile(shape, dtype, tag=name)
                nc.sync.dma_start(t[:], dram[:])
                return t
            Wasv = cload("Wasv", Wasv_d, [128, 256], FP16)
            Wdst = cload("Wdst", Wdst_d, [128, 128], FP16)
            bias16 = cload("bias16", bias_d, [128, 128], FP16)
            id16 = cload("id16", id16_d, [128, 128], FP16)
            ident_f32 = cload("ident", ident_d, [128, 128], F32)
            outW = cload("outW", outW_d, [128, 128], FP16)
            outb = cload("outb", outb_d, [128, 1], F32)
            phW = cload("phW", phW_d, [128, P_OUT], FP16)
            phb16 = cload("phb16", phb_d, [128, P_OUT], FP16)
            def cload_act(name, dram, shape, dtype):
                t = pc.tile(shape, dtype, tag=name)
                nc.scalar.dma_start(t[:], dram[:])
                return t
            gw = [cload_act(f"gw{s}", gw_d[s], [128, int(L[s]) // 16], I16)
                  for s in range(2)]
            dw = [cload_act(f"dw{s}", dw_d[s], [128, int(L[s]) // 16], I16)
                  for s in range(2)]
            zer = pc.tile([128, 128], FP16, tag="zer")
            nc.vector.memset(zer[:], 0.0)
            nc.sync.dma_start(adst_d[NBLK * 128:ADST_ROWS, :], zer[:])

            # ---------------- phase A: node tables ----------------
            asv_re = asv_d[:].rearrange("(i p) f -> p i f", p=128)
            A_TILES = NPAD // 128
            cp_engines = [nc.vector, nc.scalar]
            with tc.tile_pool(name="pa_in", bufs=3) as pa_in, \
                 tc.tile_pool(name="pa_ps", bufs=4, space="PSUM") as pa_ps, \
                 tc.tile_pool(name="pa_out", bufs=3) as pa_out:
                for t0 in range(0, A_TILES, A_CH):
                    ac = min(A_CH, A_TILES - t0)
                    hc = pa_in.tile([128, ac, 128], FP16, tag="hc")
                    nc.sync.dma_start(hc[:], hT_d[:, t0 * 128:(t0 + ac) * 128]
                                      .rearrange("p (i n) -> p i n", i=ac))
                    ao = pa_out.tile([128, ac, 256], FP16, tag="ao")
                    for i in range(0, ac, 2):
                        i2 = min(2, ac - i)
                        mm = pa_ps.tile([128, 2, 256], F32, tag="mm")
                        for ii in range(i2):
                            nc.tensor.matmul(mm[:, ii, :], hc[:, i + ii, :],
                                             Wasv[:], start=True, stop=True)
                        if (i // 2) % 2 == 0:
                            nc.vector.tensor_copy(ao[:, i:i + i2, :],
                                                  mm[:, 0:i2, :])
                        else:
                            nc.scalar.activation(ao[:, i:i + i2, :],
                                                 mm[:, 0:i2, :], ACTF.Copy)
                    w_eng = [nc.sync, nc.sync, nc.scalar][(t0 // A_CH) % 3]
                    w_eng.dma_start(asv_re[:, t0:t0 + ac, :], ao[:])
                adst_re = adst_d[0:NBLK * 128, :].rearrange(
                    "(i p) f -> p i f", p=128)
                for t0 in range(0, NBLK, A_CH):
                    ac = min(A_CH, NBLK - t0)
                    hc = pa_in.tile([128, ac, 128], FP16, tag="hc")
                    nc.sync.dma_start(hc[:], hloc_d[:, t0 * 128:(t0 + ac) * 128]
                                      .rearrange("p (i n) -> p i n", i=ac))
                    ao = pa_out.tile([128, ac, 128], FP16, tag="ao")
                    for i in range(ac):
                        mm = pa_ps.tile([128, 128], F32, tag="mm")
                        nc.tensor.matmul(mm[:], hc[:, i, :], Wdst[:],
                                         start=True, stop=False)
                        nc.tensor.matmul(mm[:], id16[:], bias16[:],
                                         start=False, stop=True)
                        if i % 2 == 0:
                            nc.vector.tensor_copy(ao[:, i, :], mm[:])
                        else:
                            nc.scalar.activation(ao[:, i, :], mm[:], ACTF.Copy)
                    nc.sync.dma_start(adst_re[:, t0:t0 + ac, :], ao[:])

            tc.strict_bb_all_engine_barrier()

            # ---------------- phase B + C ----------------
            asv_u64 = [asv_d[0:split, :].bitcast(F32),
                       asv_d[split:NPAD, :].bitcast(F32)]
            adst_u64 = adst_d[:].bitcast(F32)
            chunk_cache = [dict(), dict()]

            with tc.tile_pool(name="pg_asv", bufs=5) as pg_asv, \
                 tc.tile_pool(name="pg_dst", bufs=4) as pg_dst, \
                 tc.tile_pool(name="pg_oh", bufs=4) as pg_oh, \
                 tc.tile_pool(name="pb_t", bufs=3) as pb_t, \
                 tc.tile_pool(name="pb_t2", bufs=3) as pb_t2, \
                 tc.tile_pool(name="pb_lg", bufs=3) as pb_lg, \
                 tc.tile_pool(name="pb_er", bufs=3) as pb_er, \
                 tc.tile_pool(name="pb_wt", bufs=4) as pb_wt, \
                 tc.tile_pool(name="ps_agg", bufs=3, space="PSUM") as ps_agg, \
                 tc.tile_pool(name="pc_ps", bufs=2, space="PSUM") as pc_ps, \
                 tc.tile_pool(name="pc_ph", bufs=1, space="PSUM") as pc_ph, \
                 tc.tile_pool(name="pc_sb", bufs=3) as pc_sb:

                def ensure_chunk(s, ci):
                    if ci in chunk_cache[s]:
                        return chunk_cache[s][ci]
                    t0 = ci * G
                    g = min(G, int(tiles[s]) - t0)
                    GSUB = 8
                    asv_g = pg_asv.tile([128, g, 128], F32, tag="asv_g")
                    dst_g = pg_dst.tile([128, g, 64], F32, tag="dst_g")
                    for k in range(0, g, GSUB):
                        gs = min(GSUB, g - k)
                        ne = gs * 128
                        nc.gpsimd.dma_gather(
                            asv_g[:, k:k + gs, :], asv_u64[s],
                            gw[s][:, (t0 + k) * 8:(t0 + k + gs) * 8], ne, ne, 128)
                        nc.gpsimd.dma_gather(
                            dst_g[:, k:k + gs, :], adst_u64,
                            dw[s][:, (t0 + k) * 8:(t0 + k + gs) * 8], ne, ne, 64)
                    av = asv_g[:].bitcast(FP16)   # [128, g, 256]
                    dv = dst_g[:].bitcast(FP16)   # [128, g, 128]
                    # dst one-hot for the scatter: host-precomputed, via SP DMA
                    ohh = pg_oh.tile([128, g, 128], FP16, tag="ohh")
                    nc.sync.dma_start(
                        ohh[:], oh_d[s][:, t0 * 128:(t0 + g) * 128]
                        .rearrange("p (i n) -> p i n", i=g))
                    # t = a_src + a_dst (+bias already in table); leaky via
                    # t2 = 0.2t (Act copy w/ scale) then per-sign min/max TT
                    q = pb_t.tile([128, g, 128], FP16, tag="q")
                    nc.vector.tensor_tensor(q[:], av[:, :, 0:128],
                                            dv[:], op=AT.add)
                    t2 = pb_t2.tile([128, g, 128], FP16, tag="t2")
                    nc.scalar.activation(t2[:], q[:], ACTF.Copy, scale=0.2)
                    for hh in range(H):
                        c = cpos[hh]
                        if c > 0:
                            sl = q[:, :, hh * 64:hh * 64 + c]
                            nc.vector.tensor_tensor(
                                sl, sl, t2[:, :, hh * 64:hh * 64 + c],
                                op=AT.max)
                        if c < 64:
                            sl = q[:, :, hh * 64 + c:(hh + 1) * 64]
                            nc.vector.tensor_tensor(
                                sl, sl, t2[:, :, hh * 64 + c:(hh + 1) * 64],
                                op=AT.min)
                    # binary folds 64->8 (fp16 TT adds) then small reduce
                    th = q[:].rearrange("p g (h d) -> p g h d", h=H)
                    fold_engs = {32: nc.gpsimd, 16: nc.vector, 8: nc.vector,
                                 4: nc.vector}
                    for w in (32, 16, 8, 4):
                        fold_engs[w].tensor_tensor(
                            th[:, :, :, 0:w], th[:, :, :, 0:w],
                            th[:, :, :, w:2 * w], op=AT.add)
                    lg = pb_lg.tile([128, g, H], F32, tag="lg")
                    nc.vector.tensor_reduce(lg[:], th[:, :, :, 0:4],
                                            axis=mybir.AxisListType.X, op=AT.add)
                    # wt = [ex-scaled values | ex]; exp into cols 128:130;
                    # narrow (width-4) exp replica, broadcast on the middle
                    # axis in the value-scale TT (keeps 2x mode)
                    wt = pb_wt.tile([128, g, 130], FP16, tag="wt")
                    nc.scalar.activation(wt[:, :, 128:130], lg[:], ACTF.Exp)
                    er = pb_er.tile([128, g, H, 4], FP16, tag="er")
                    nc.scalar.activation(
                        er[:],
                        lg[:].rearrange("p g (h o) -> p g h o", o=1)
                        .to_broadcast((128, g, H, 4)), ACTF.Exp)
                    wtv = wt[:, :, 0:128].rearrange(
                        "p g (h o f) -> p g h o f", h=H, o=16)
                    avv = av[:, :, 128:256].rearrange(
                        "p g (h o f) -> p g h o f", h=H, o=16)
                    erb = er[:].rearrange("p g h (o f) -> p g h o f", o=1) \
                        .to_broadcast((128, g, H, 16, 4))
                    nc.vector.tensor_tensor(wtv[:, :, 0:1], avv[:, :, 0:1],
                                            erb[:, :, 0:1], op=AT.mult)
                    nc.vector.tensor_tensor(wtv[:, :, 1:2, 0:4],
                                            avv[:, :, 1:2, 0:4],
                                            erb[:, :, 1:2, 0:4], op=AT.mult)
                    nc.gpsimd.tensor_tensor(wtv[:, :, 1:2, 4:16],
                                            avv[:, :, 1:2, 4:16],
                                            erb[:, :, 1:2, 4:16], op=AT.mult)
                    chunk_cache[s][ci] = (ohh, wt)
                    return ohh, wt

                for j in range(NBLK):
                    n_ev = int(T[0][j] + T[1][j])
                    ps = ps_agg.tile([128, 130], F32, tag="ps")
                    ev = 0
                    for s in range(2):
                        for t_ in range(int(T[s][j])):
                            gt = int(base_tile[s, j]) + t_
                            ohh, wt = ensure_chunk(s, gt // G)
                            off = gt % G
                            nc.tensor.matmul(ps[:], ohh[:, off, :],
                                             wt[:, off, 0:130],
                                             start=(ev == 0),
                                             stop=(ev == n_ev - 1))
                            ev += 1
                    # ---- phase C for block j ----
                    R = 128 if j < NBLK - 1 else last_rows
                    rc = pc_sb.tile([128, 2], F32, tag="rc")
                    nc.vector.reciprocal(rc[:], ps[:, 128:130])
                    agg = pc_sb.tile([128, 128], F32, tag="agg")
                    if R < 128:
                        nc.vector.memset(agg[:], 0.0)
                    nc.scalar.activation(agg[0:R, 0:64], ps[0:R, 0:64],
                                         ACTF.Copy, scale=rc[0:R, 0:1])
                    nc.scalar.activation(agg[0:R, 64:128], ps[0:R, 64:128],
                                         ACTF.Copy, scale=rc[0:R, 1:2])
                    tp = pc_ps.tile([128, 128], F32, tag="tp")
                    nc.tensor.transpose(tp[:], agg[:], ident_f32[:])
                    aggT = pc_sb.tile([128, 128], FP16, tag="aggT")
                    nc.scalar.activation(aggT[:], tp[:], ACTF.Copy)
                    o1p = pc_ps.tile([128, 128], F32, tag="o1p")
                    nc.tensor.matmul(o1p[:], outW[:], aggT[:], start=True,
                                     stop=True)
                    o1 = pc_sb.tile([128, 128], FP16, tag="o1")
                    nc.scalar.activation(o1[:], o1p[:], ACTF.Relu,
                                         bias=outb[:, 0:1])
                    php = pc_ph.tile([128, P_OUT], F32, tag="php")
                    nc.tensor.matmul(php[:], o1[:], phW[:], start=True,
                                     stop=False)
                    nc.tensor.matmul(php[:], id16[:], phb16[:], start=False,
                                     stop=True)
                    ez = pc_sb.tile([128, P_OUT], F32, tag="ez")
                    nc.scalar.activation(ez[:], php[:], ACTF.Exp)
                    sm = pc_sb.tile([128, 1], F32, tag="sm")
                    nc.vector.tensor_reduce(sm[:], ez[:],
                                            axis=mybir.AxisListType.X, op=AT.add)
                    rc2 = pc_sb.tile([128, 1], F32, tag="rc2")
                    nc.vector.reciprocal(rc2[:], sm[:])
                    ot = pc_sb.tile([128, P_OUT], F32, tag="ot")
                    nc.vector.tensor_scalar(ot[:], ez[:], rc2[:, 0:1], None,
                                            op0=AT.mult)
                    nc.sync.dma_start(out_d[j * 128:j * 128 + R, :], ot[0:R, :])

    nc.compile()
    return nc


_CACHE = {}


def kernel(**inputs) -> np.ndarray:
    meta, in_maps = prep(**inputs)
    key = "nc"
    if key not in _CACHE:
        _CACHE[key] = build(meta)
    nc = _CACHE[key]
    res = run_bass_kernel_spmd(nc, in_maps, core_ids=list(range(N_CORES)))
    out = np.concatenate([res.results[c]["out"] for c in range(N_CORES)],
                         axis=0)
    return out.astype(np.float32)
